# revision 33
# baseline (speedup 1.0000x reference)
"""Trainium2 Bass kernel for the CoSSM block (bidirectional Mamba on two streams).

Sharding: 8 cores = 2 streams x 4 batches; each core runs the full pipeline for
its (stream, batch) slice.  Weights are replicated; the per-core input map
carries the stream-specific resbranch/LN weights.

On-chip layout is channel-major ([channels(partitions) x L(free)]).

v3 design notes (engine balance):
- fp16 scan pipeline: dbu / C-mult run as packed 2x-mode TensorTensor on DVE,
  the scans run packed (4 x 513 segments with zero reset columns) mostly on
  the Pool (gpsimd) engine with a slice diverted to DVE for balance.
- The n-reduction (sum of 16 state contributions) and the u*D skip term ride
  the PE: identity / diag(D) matmuls accumulating into one PSUM bank.
- B/C state-row broadcasts go DRAM->SBUF via stride-0-partition DMA.
- in_proj / dwconv / out_proj weights are fp16 (1 PE cycle/row).
- da = exp(-n*delta) on Act; da_2 = da_1^2 on DVE to balance Act/DVE.
- Small per-core constants ride in one packed DMA; activation-table choice is
  patched so Exp and Ln share one table (no per-dc table thrash).
"""

import sys
import types
import numpy as np

for _p in ("/opt/trn_rl_repo",):
    if _p not in sys.path:
        sys.path.append(_p)

import concourse.mybir as mybir
from concourse import bacc
from concourse.tile import TileContext
from concourse.bass_utils import run_bass_kernel_spmd

F32 = mybir.dt.float32
F16 = mybir.dt.float16
F32R = mybir.dt.float32r
AL = mybir.AluOpType
AF = mybir.ActivationFunctionType
AX = mybir.AxisListType

B, L = 4, 512
D_IN, D_MODEL = 256, 1024
D_INNER, D_STATE, D_CONV, DT_RANK = 2048, 16, 4, 64
BN_EPS, LN_EPS = 1e-5, 1e-6

NDC = D_INNER // 128          # 16 channel chunks of d_inner
NMC = D_MODEL // 128          # 8 channel chunks of d_model
NG = 4                        # states packed per scan instruction
NGRP = D_STATE // NG          # 4 groups
SEG = L + 1                   # segment length incl. reset column

# consts tile column layout
C_RBB, C_LNG, C_LNB = 0, 8, 16
C_CB, C_DTB, C_DV = 24, 56, 88
C_AN = 120                    # + br*256 + dc*16 + n   (512 cols)
C_TOT = 632

SCAN_DVE_MOD = 12             # every 12th (dc,g) scan runs on DVE instead of Pool

_CACHE = {}


def _patch_act_tables(nc):
    """Make Exp and Ln resolve to the combined exp+ln activation table so the
    per-dc Exp/Ln alternation doesn't reload tables 60+ times."""
    from concourse.hw_specs import get_activation_tables
    import bass_rust as _bass_rust

    def insert_act_table_loads(self):
        has_activation = any(
            isinstance(i, mybir.InstActivation)
            for b in self.main_func.blocks
            for i in b.instructions
        )
        if not has_activation:
            return
        tables = []
        items = list(get_activation_tables(self.m.arch).items())
        combined = {
            name for name, s in items
            if AF.Exp in s and AF.Ln in s
        }
        for name, s in items:
            if name not in combined and (AF.Exp in s) != (AF.Ln in s):
                s = s - {AF.Exp, AF.Ln}
            tables.append((name, s))
        _bass_rust.insert_act_table_loads(self, tables)

    nc.insert_act_table_loads = types.MethodType(insert_act_table_loads, nc)


def _build_program():
    nc = bacc.Bacc("TRN2", target_bir_lowering=False, debug=False)
    _patch_act_tables(nc)

    def din(name, shape, dt=F32):
        return nc.dram_tensor(name, list(shape), dt, kind="ExternalInput")

    d_x = din("x", [4, 128, D_IN], F32R)
    d_identr = din("identr", [128, 128], F32R)
    d_identf = din("identf", [128, 128], F32)
    d_identh = din("identh", [128, 128], F16)
    d_ones = din("ones", [1, 128], F32R)
    d_onescol = din("onescol", [128, 1], F16)
    d_rbw = din("rbw", [6, 128, D_MODEL], F32R)       # (k,cc) major
    d_rbskip = din("rbskip", [2, 128, D_MODEL], F32R)
    d_wproj = din("wproj", [8, 128, 2 * D_INNER], F16)
    d_convd = din("convd", [2, 4, 128, 4 * 512], F16)  # 4-dc batches
    d_xpw = din("xpw", [2, NDC, 128, 112], F16)
    d_dtw = din("dtw", [2, 2, 64, 1024], F32R)         # per-branch halves
    d_opw = din("opw", [NDC, 128, D_MODEL], F16)
    d_dvdiag = din("dvdiag", [128, 2 * NDC * 128], F16)
    d_consts = din("consts", [128, C_TOT], F32)

    d_out = nc.dram_tensor("y_out", [L, D_MODEL], F16, kind="ExternalOutput")
    # DRAM spill buffers (per-core scratch)
    d_x1 = nc.dram_tensor("x1_spill", [NMC, 128, L], F16)
    d_z = nc.dram_tensor("z_spill", [NDC, 128, L], F16)
    d_bc = nc.dram_tensor("bc_spill", [2, 2, D_STATE, L], F16)

    with TileContext(nc) as tc:
        with tc.tile_pool(name="sb", bufs=1) as sb, \
             tc.tile_pool(name="wp", bufs=3) as wp, \
             tc.tile_pool(name="tp", bufs=2) as tp, \
             tc.tile_pool(name="ps", bufs=8, space="PSUM") as ps:

            def psum():
                return ps.tile([128, 512], F32, tag="mm", name="mm")

            # ---- constants -------------------------------------------------
            t_cst = sb.tile([128, C_TOT], F32, tag="cst", name="cst")
            nc.sync.dma_start(out=t_cst[:], in_=d_consts[:])

            def an(br, dc, n):
                c = C_AN + br * 256 + dc * 16 + n
                return t_cst[:, c:c + 1]

            t_idr = sb.tile([128, 128], F32R, tag="idr", name="idr")
            nc.sync.dma_start(out=t_idr[:], in_=d_identr[:])
            t_idf = sb.tile([128, 128], F32, tag="idf", name="idf")
            nc.sync.dma_start(out=t_idf[:], in_=d_identf[:])
            t_idh = sb.tile([128, 128], F16, tag="idh", name="idh")
            nc.sync.dma_start(out=t_idh[:], in_=d_identh[:])
            t_ones = sb.tile([1, 128], F32R, tag="ones", name="ones")
            nc.sync.dma_start(out=t_ones[:], in_=d_ones[:])
            t_onescol = sb.tile([128, 1], F16, tag="onescol", name="onescol")
            nc.sync.dma_start(out=t_onescol[:], in_=d_onescol[:])
            t_dgall = sb.tile([128, 2 * NDC * 128], F16, tag="dgall", name="dgall")
            nc.sync.dma_start(out=t_dgall[:], in_=d_dvdiag[:])

            # ---- phase A: load x, transpose to [c, l] ----------------------
            t_xpad = [sb.tile([128, 514], F32R, tag=f"xp{cc}", name=f"xp{cc}") for cc in range(2)]
            for cc in range(2):
                nc.vector.memset(t_xpad[cc][:].bitcast(F32), 0.0)
            for i in range(4):
                xt = tp.tile([128, D_IN], F32R, tag="xinz", name="xin", bufs=2)
                nc.sync.dma_start(out=xt[:], in_=d_x[i])
                for cc in range(2):
                    pt = ps.tile([128, 128], F32R, tag="mm", name="mm")
                    nc.tensor.transpose(pt[:], xt[:, cc * 128:(cc + 1) * 128], t_idr[:])
                    nc.scalar.copy(t_xpad[cc][:, 1 + i * 128:1 + (i + 1) * 128], pt[:])

            # ---- phase B: resbranch -> x1 (fp16) ---------------------------
            # conv3 pass: 8 psum accumulators, one weight tile live at a time
            t_x1 = [sb.tile([128, L], F16, tag=f"x1_{m}", name=f"x1_{m}") for m in range(NMC)]
            pts = [psum() for _ in range(NMC)]
            for kc in range(6):
                k, cc = kc // 2, kc % 2
                wt = wp.tile([128, 1024], F32R, tag="wbig", name="wbig", bufs=4)
                nc.sync.dma_start(out=wt[:], in_=d_rbw[kc])
                for m in range(NMC):
                    nc.tensor.matmul(pts[m][:], wt[:, m * 128:(m + 1) * 128],
                                     t_xpad[cc][:, k:k + 512],
                                     start=(kc == 0), stop=(kc == 5))
            for m in range(NMC):
                nc.scalar.activation(t_x1[m][:], pts[m][:], AF.Relu,
                                     bias=t_cst[:, C_RBB + m:C_RBB + m + 1])
            # 1x1 skip pass, then add in place
            pts2 = [psum() for _ in range(NMC)]
            for cc in range(2):
                wt = wp.tile([128, 1024], F32R, tag="wbig", name="wbig", bufs=4)
                nc.sync.dma_start(out=wt[:], in_=d_rbskip[cc])
                for m in range(NMC):
                    nc.tensor.matmul(pts2[m][:], wt[:, m * 128:(m + 1) * 128],
                                     t_xpad[cc][:, 1:513],
                                     start=(cc == 0), stop=(cc == 1))
            for m in range(NMC):
                nc.vector.tensor_tensor(out=t_x1[m][:], in0=t_x1[m][:], in1=pts2[m][:], op=AL.add)
                nc.sync.dma_start(out=d_x1[m], in_=t_x1[m][:])

            # ---- phase C: in_proj (fp16) -> xi_pad fp16, z spill fp16 ------
            t_xi = [sb.tile([128, 518], F16, tag=f"xi{dc}", name=f"xi{dc}") for dc in range(NDC)]
            for dc in range(NDC):
                nc.vector.memset(t_xi[dc][:].bitcast(F32), 0.0)
            t_u = [[None] * NDC, [None] * NDC]

            def inproj_group(g, zcopy_pool=False):
                pts = [psum() for _ in range(8)]
                for kc in range(NMC):
                    wt = wp.tile([128, 1024], F16, tag="wbig", name="wbig", bufs=4)
                    nc.sync.dma_start(out=wt[:], in_=d_wproj[kc][:, g * 1024:(g + 1) * 1024])
                    for mj in range(8):
                        nc.tensor.matmul(pts[mj][:], wt[:, mj * 128:(mj + 1) * 128],
                                         t_x1[kc][:], start=(kc == 0), stop=(kc == NMC - 1))
                for mj in range(8):
                    mm = g * 8 + mj
                    if mm < NDC:
                        nc.scalar.copy(t_xi[mm][:, 3:515], pts[mj][:])
                    else:
                        zt = tp.tile([128, L], F16, tag="ztmp", name="ztmp")
                        nc.scalar.activation(zt[:], pts[mj][:], AF.Silu)
                        nc.sync.dma_start(out=d_z[mm - NDC], in_=zt[:])

            def dwconv_batch(br, dq):
                # depthwise conv + silu -> u (fp16) for dc in [4*dq, 4*dq+4)
                cdt4 = wp.tile([128, 2048], F16, tag="wbig", name="cdt4", bufs=4)
                nc.sync.dma_start(out=cdt4[:], in_=d_convd[br, dq])
                for dj in range(4):
                    dc = dq * 4 + dj
                    pu = psum()
                    for k in range(4):
                        if br == 0:
                            rhs = t_xi[dc][:, k:k + 512]
                        else:
                            rhs = t_xi[dc][:, 6 - k:518 - k][:, ::-1]
                        nc.tensor.matmul(pu[:], cdt4[:, dj * 512 + k * 128:dj * 512 + (k + 1) * 128],
                                         rhs, start=(k == 0), stop=(k == 3))
                    ut = sb.tile([128, L], F16, tag=f"u{br}_{dc}", name=f"u{br}_{dc}")
                    nc.scalar.activation(ut[:], pu[:], AF.Silu,
                                         bias=t_cst[:, C_CB + dc * 2 + br:C_CB + dc * 2 + br + 1])
                    t_u[br][dc] = ut

            # ---- phases C+D interleaved: in_proj groups feed dwconv early --
            inproj_group(0)
            for br in range(2):
                for dq in (0, 1):
                    dwconv_batch(br, dq)
            inproj_group(1)
            for br in range(2):
                for dq in (2, 3):
                    dwconv_batch(br, dq)

            # y accumulators (fp16, in xi slots; filled per branch below)
            t_y = [sb.tile([128, L], F16, tag=f"xi{dc}", name=f"y{dc}") for dc in range(NDC)]

            # ---- phase E: selective scan per branch ------------------------
            t_Bb = [sb.tile([128, NG * L], F16, tag=f"Bb{g}", name=f"Bb{g}") for g in range(NGRP)]
            t_Cb = [sb.tile([128, NG * L], F16, tag=f"Cb{g}", name=f"Cb{g}") for g in range(NGRP)]
            n_da = [0]
            n_flex = [0]

            def pool_pick():
                # route 7 of every 16 dbu/q TensorTensors to the Pool ucode
                r = (n_flex[0] * 7) % 16 < 7
                n_flex[0] += 1
                return r

            for br in range(2):
                # x_proj: px [112, L] = dt rows 0:64, B rows 64:80, C rows 96:112
                px = ps.tile([112, 512], F32, tag="mm", name="px")
                for dh in range(2):
                    wx = wp.tile([128, 8 * 112], F16, tag="wxp", name="wxp", bufs=2)
                    nc.sync.dma_start(out=wx[:].rearrange("p (d c) -> p d c", d=8),
                                      in_=d_xpw[br, dh * 8:(dh + 1) * 8].transpose([1, 0, 2]))
                    for dj in range(8):
                        dc = dh * 8 + dj
                        nc.tensor.matmul(px[:], wx[:, dj * 112:(dj + 1) * 112],
                                         t_u[br][dc][:],
                                         start=(dc == 0), stop=(dc == NDC - 1))
                t_dtT = sb.tile([64, L], F32R, tag="xp0", name="dtT")
                nc.scalar.copy(t_dtT[:], px[0:64, :])
                t_Brow = tp.tile([D_STATE, L], F16, tag="Brow", name="Brow", bufs=1)
                nc.scalar.copy(t_Brow[:], px[64:80, :])
                t_Crow = tp.tile([D_STATE, L], F16, tag="Crow", name="Crow", bufs=1)
                nc.scalar.copy(t_Crow[:], px[96:112, :])
                nc.sync.dma_start(out=d_bc[br, 0], in_=t_Brow[:])
                nc.sync.dma_start(out=d_bc[br, 1], in_=t_Crow[:])
                for g in range(NGRP):
                    src_b = d_bc[br, 0, 4 * g:4 * g + 4].rearrange("n c -> (n c)").unsqueeze(0).to_broadcast([128, NG * L])
                    nc.sync.dma_start(out=t_Bb[g][:], in_=src_b)
                    src_c = d_bc[br, 1, 4 * g:4 * g + 4].rearrange("n c -> (n c)").unsqueeze(0).to_broadcast([128, NG * L])
                    nc.sync.dma_start(out=t_Cb[g][:], in_=src_c)
                t_dtw = []
                for h in range(2):
                    wdt = wp.tile([64, 1024], F32R, tag="wdt", name="wdt", bufs=4)
                    nc.sync.dma_start(out=wdt[:], in_=d_dtw[br, h])
                    t_dtw.append(wdt)

                def delta(dc):
                    # softplus(dt_proj) for chunk dc -> (d8, w8), fp16
                    pd = psum()
                    nc.tensor.matmul(pd[:], t_dtw[dc // 8][:, (dc % 8) * 128:(dc % 8 + 1) * 128],
                                     t_dtT[:], start=True, stop=True)
                    et = tp.tile([128, L], F16, tag="eth", name="et", bufs=4)
                    nc.scalar.activation(et[:], pd[:], AF.Exp,
                                         bias=t_cst[:, C_DTB + dc * 2 + br:C_DTB + dc * 2 + br + 1])
                    d8 = tp.tile([128, L], F16, tag="d8", name="d8", bufs=4)
                    nc.scalar.activation(d8[:], et[:], AF.Ln, bias=1.0)
                    w8 = tp.tile([128, L], F16, tag="w8", name="w8", bufs=4)
                    nc.vector.tensor_tensor(out=w8[:], in0=d8[:],
                                            in1=t_u[br][dc][:], op=AL.mult)
                    return d8, w8

                # Software-pipelined scan loop: scans are consumed (C-mult +
                # PE reduction) with a LAG of 2 (dc,g)-iterations so the DVE
                # always has dbu work queued ahead of q's that wait on Pool.
                LAG = 2
                paccs = {}
                pend = []

                def consume(hh0, dc0, g0):
                    q = tp.tile([128, NG * L], F16, tag="q", name="q", bufs=2)
                    eng = nc.gpsimd if pool_pick() else nc.vector
                    eng.tensor_tensor(
                        out=q[:].rearrange("p (n c) -> p n c", n=NG),
                        in0=hh0[:].rearrange("p (n c) -> p n c", n=NG)[:, :, 1:SEG],
                        in1=t_Cb[g0][:].rearrange("p (n c) -> p n c", n=NG),
                        op=AL.mult)
                    for jn in range(NG):
                        qs = q[:, jn * L:(jn + 1) * L]
                        if br == 1:
                            qs = qs[:, ::-1]
                        nc.tensor.matmul(paccs[dc0][:], t_idh[:], qs,
                                         start=False,
                                         stop=(g0 == NGRP - 1 and jn == NG - 1))
                    if g0 == NGRP - 1:
                        if br == 0:
                            nc.vector.tensor_copy(out=t_y[dc0][:], in_=paccs[dc0][:])
                        else:
                            nc.vector.tensor_tensor(out=t_y[dc0][:], in0=t_y[dc0][:],
                                                    in1=paccs[dc0][:], op=AL.add)
                        del paccs[dc0]

                dq_delta = [delta(0), delta(1), delta(2)]
                # z-half of in_proj rides the pipeline-fill windows (PE slack);
                # silu is applied at copy time so phase F only multiplies.
                inproj_group(2 + br)
                for dc in range(NDC):
                    d8, w8 = dq_delta.pop(0)
                    # PSUM accumulator for this (br, dc): u*D skip + 16 state slices
                    pacc = psum()
                    paccs[dc] = pacc
                    mv = t_u[br][dc][:] if br == 0 else t_u[br][dc][:, ::-1]
                    nc.tensor.matmul(pacc[:], t_dgall[:, (br * NDC + dc) * 128:(br * NDC + dc + 1) * 128],
                                     mv, start=True, stop=False)
                    for g in range(NGRP):
                        if dc + 3 < NDC and g == 2:
                            dq_delta.append(delta(dc + 3))
                        da = tp.tile([128, NG * SEG], F16, tag="da", name="da", bufs=3)
                        dbu = tp.tile([128, NG * SEG], F16, tag="dbu", name="dbu", bufs=2)
                        if n_da[0] < 3:
                            nc.vector.memset(da[:].bitcast(F32), 0.0)
                        if n_da[0] < 2:
                            nc.vector.memset(dbu[:].bitcast(F32), 0.0)
                        for jn in range(NG):
                            n = g * NG + jn
                            nc.scalar.activation(
                                da[:, jn * SEG + 1:(jn + 1) * SEG], d8[:],
                                AF.Exp, scale=an(br, dc, n))
                        dbu_sl = dbu[:].rearrange("p (n c) -> p n c", n=NG)[:, :, 1:SEG]
                        eng = nc.gpsimd if pool_pick() else nc.vector
                        eng.tensor_tensor(
                            out=dbu_sl,
                            in0=w8[:].unsqueeze(1).to_broadcast([128, NG, L]),
                            in1=t_Bb[g][:].rearrange("p (n c) -> p n c", n=NG),
                            op=AL.mult)
                        hh = tp.tile([128, NG * SEG], F16, tag="hh", name="hh", bufs=3)
                        nc.vector.tensor_tensor_scan(hh[:], da[:], dbu[:], 0.0,
                                                     AL.mult, AL.add)
                        n_da[0] += 1
                        pend.append((hh, dc, g))
                        if len(pend) > LAG:
                            consume(*pend.pop(0))
                while pend:
                    consume(*pend.pop(0))

            # ---- phase F: gate, out_proj, layernorm, residual --------------
            for dc in range(NDC):
                zt = tp.tile([128, L], F16, tag="ztmp", name="ztmp")
                nc.sync.dma_start(out=zt[:], in_=d_z[dc])
                nc.vector.tensor_tensor(out=t_y[dc][:], in0=t_y[dc][:], in1=zt[:], op=AL.mult)

            t_o1 = [sb.tile([128, L], F16, tag=f"u0_{m}", name=f"o1_{m}") for m in range(NMC)]
            pos = [psum() for _ in range(NMC)]
            for dp in range(NDC // 2):
                wt = wp.tile([128, 2048], F16, tag="wbig", name="wbig", bufs=4)
                nc.sync.dma_start(out=wt[:].rearrange("p (d c) -> p d c", d=2),
                                  in_=d_opw[2 * dp:2 * dp + 2].transpose([1, 0, 2]))
                for dj in range(2):
                    dc = 2 * dp + dj
                    for m in range(NMC):
                        nc.tensor.matmul(pos[m][:], wt[:, dj * 1024 + m * 128:dj * 1024 + (m + 1) * 128],
                                         t_y[dc][:], start=(dc == 0), stop=(dc == NDC - 1))
            for m in range(NMC):
                nc.scalar.copy(t_o1[m][:], pos[m][:])

            # layernorm stats via column-sum matmuls
            pm = ps.tile([1, 512], F32, tag="mm", name="pm")
            for m in range(NMC):
                nc.tensor.matmul(pm[:], t_onescol[:], t_o1[m][:],
                                 start=(m == 0), stop=(m == NMC - 1))
            pq = ps.tile([1, 512], F32, tag="mm", name="pq")
            for m in range(NMC):
                sq = tp.tile([128, L], F16, tag="ztmp", name="sq")
                nc.scalar.activation(sq[:], t_o1[m][:], AF.Square)
                nc.tensor.matmul(pq[:], t_onescol[:], sq[:],
                                 start=(m == 0), stop=(m == NMC - 1))
            t_mean = sb.tile([1, L], F32R, tag="mean", name="mean")
            nc.scalar.activation(t_mean[:], pm[:], AF.Copy, scale=1.0 / D_MODEL)
            t_var = tp.tile([1, L], F32, tag="et", name="stat")
            nc.scalar.activation(t_var[:], pq[:], AF.Copy, scale=1.0 / D_MODEL)
            msq = tp.tile([1, L], F32, tag="et", name="msq")
            nc.vector.tensor_tensor(out=msq[:], in0=t_mean[:], in1=t_mean[:], op=AL.mult)
            nc.vector.tensor_tensor(out=t_var[:], in0=t_var[:], in1=msq[:], op=AL.subtract)
            t_eps = sb.tile([1, 1], F32, tag="eps", name="eps")
            nc.vector.memset(t_eps[:], LN_EPS)
            t_sd = tp.tile([1, L], F32, tag="q", name="stat2", bufs=2)
            nc.scalar.activation(t_sd[:], t_var[:], AF.Sqrt, bias=t_eps[:])
            t_isd = sb.tile([1, L], F32R, tag="isd", name="isd")
            with nc.allow_low_precision(reason="isd is a broadcast-matmul rhs"):
                nc.vector.reciprocal(out=t_isd[:], in_=t_sd[:])
            # broadcast mean, isd
            pmb = psum()
            nc.tensor.matmul(pmb[:], t_ones[:], t_mean[:], start=True, stop=True)
            t_mb = sb.tile([128, L], F32, tag="Bb0", name="mb")
            nc.scalar.copy(t_mb[:], pmb[:])
            pib = psum()
            nc.tensor.matmul(pib[:], t_ones[:], t_isd[:], start=True, stop=True)
            t_ib = sb.tile([128, L], F32, tag="Bb1", name="ib")
            nc.scalar.copy(t_ib[:], pib[:])

            t_of = []
            for m in range(NMC):
                x1r = tp.tile([128, L], F16, tag="d8", name="x1r", bufs=4)
                nc.sync.dma_start(out=x1r[:], in_=d_x1[m])
                tt = tp.tile([128, L], F32, tag="et", name="ft")
                nc.vector.tensor_tensor(out=tt[:], in0=t_o1[m][:], in1=t_mb[:], op=AL.subtract)
                nc.vector.tensor_tensor(out=tt[:], in0=tt[:], in1=t_ib[:], op=AL.mult)
                nc.vector.tensor_scalar(out=tt[:], in0=tt[:],
                                        scalar1=t_cst[:, C_LNG + m:C_LNG + m + 1],
                                        scalar2=t_cst[:, C_LNB + m:C_LNB + m + 1],
                                        op0=AL.mult, op1=AL.add)
                ot = sb.tile([128, L], F16, tag=f"u1_{m}", name=f"of_{m}")
                nc.vector.tensor_tensor(out=ot[:], in0=tt[:], in1=x1r[:], op=AL.add)
                t_of.append(ot)

            # transpose back to [l, d] and store
            for i in range(4):
                outt = wp.tile([128, D_MODEL], F16, tag="wbig", name="outt", bufs=4)
                for m in range(NMC):
                    ptr = ps.tile([128, 128], F16, tag="mm", name="ptr")
                    nc.tensor.transpose(ptr[:], t_of[m][:, i * 128:(i + 1) * 128], t_idh[:])
                    nc.vector.tensor_copy(out=outt[:, m * 128:(m + 1) * 128], in_=ptr[:])
                nc.sync.dma_start(out=d_out[i * 128:(i + 1) * 128, :], in_=outt[:])

    nc.compile()
    return nc


def _prep_core_inputs(x, rb_conv_w, rb_bn_g, rb_bn_b, rb_skip_w, inp, ln_g, ln_b):
    f32 = np.float32
    f16 = np.float16
    out = {}
    out["x"] = np.ascontiguousarray(x.reshape(4, 128, D_IN)).astype(f32)
    out["identr"] = np.eye(128, dtype=f32)
    out["identf"] = np.eye(128, dtype=f32)
    out["identh"] = np.eye(128, dtype=f16)
    out["ones"] = np.ones((1, 128), f32)
    out["onescol"] = np.ones((128, 1), f16)
    s = f32(1.0) / np.sqrt(np.float64(1.0 + BN_EPS))
    Wc = (rb_conv_w * (rb_bn_g * s)[:, None, None]).astype(f32)   # [1024,256,3]
    rbw = np.transpose(Wc, (2, 1, 0)).reshape(6, 128, D_MODEL)
    out["rbw"] = np.ascontiguousarray(rbw)
    rbs = rb_skip_w[:, :, 0].T.reshape(2, 128, D_MODEL)           # [c, m]
    out["rbskip"] = np.ascontiguousarray(rbs.astype(f32))
    out["wproj"] = np.ascontiguousarray(inp["in_proj_w"].T.reshape(8, 128, 2 * D_INNER).astype(f16))
    convd = np.zeros((2, NDC, 128, 4, 128), f16)
    r = np.arange(128)
    for br, key in enumerate(["conv_w_f", "conv_w_b"]):
        cw = inp[key].astype(f16)
        for dc in range(NDC):
            for k in range(4):
                convd[br, dc, r, k, r] = cw[dc * 128:(dc + 1) * 128, k]
    out["convd"] = np.ascontiguousarray(convd.reshape(2, 4, 4, 128, 512).transpose(0, 1, 3, 2, 4).reshape(2, 4, 128, 2048))
    xpw = np.zeros((2, D_INNER, 112), np.float32)
    for br, key in enumerate(["x_proj_f", "x_proj_b"]):
        xp = inp[key].T  # [2048, 96]
        xpw[br, :, 0:80] = xp[:, 0:80]
        xpw[br, :, 96:112] = xp[:, 80:96]
    out["xpw"] = np.ascontiguousarray(xpw.reshape(2, NDC, 128, 112).astype(f16))
    out["dtw"] = np.ascontiguousarray(np.stack(
        [inp["dt_w_f"].T, inp["dt_w_b"].T]).reshape(2, 64, 2, 1024).transpose(0, 2, 1, 3).astype(f32))
    out["opw"] = np.ascontiguousarray(inp["out_proj_w"].T.reshape(NDC, 128, D_MODEL).astype(f16))
    dvd = np.zeros((2, NDC, 128, 128), f16)
    for br, key in enumerate(["D_f", "D_b"]):
        dv = inp[key].astype(f16).reshape(NDC, 128)
        for dc in range(NDC):
            dvd[br, dc, r, r] = dv[dc]
    out["dvdiag"] = np.ascontiguousarray(dvd.transpose(2, 0, 1, 3).reshape(128, 2 * NDC * 128))

    cst = np.zeros((128, C_TOT), f32)
    cst[:, C_RBB:C_RBB + 8] = rb_bn_b.reshape(NMC, 128).T
    cst[:, C_LNG:C_LNG + 8] = ln_g.reshape(NMC, 128).T
    cst[:, C_LNB:C_LNB + 8] = ln_b.reshape(NMC, 128).T
    cb = np.stack([inp["conv_bias_f"].reshape(NDC, 128),
                   inp["conv_bias_b"].reshape(NDC, 128)], -1)     # [16,128,2]
    cst[:, C_CB:C_CB + 32] = cb.transpose(1, 0, 2).reshape(128, 32)
    dtb = np.stack([inp["dt_bias_f"].reshape(NDC, 128),
                    inp["dt_bias_b"].reshape(NDC, 128)], -1)
    cst[:, C_DTB:C_DTB + 32] = dtb.transpose(1, 0, 2).reshape(128, 32)
    dvv = np.stack([inp["D_f"].reshape(NDC, 128), inp["D_b"].reshape(NDC, 128)], -1)
    cst[:, C_DV:C_DV + 32] = dvv.transpose(1, 0, 2).reshape(128, 32)
    aneg = np.stack([-np.exp(inp["A_log_f"]), -np.exp(inp["A_log_b"])])  # [2,2048,16]
    cst[:, C_AN:C_AN + 512] = aneg.reshape(2, NDC, 128, 16).transpose(2, 0, 1, 3).reshape(128, 512)
    out["consts"] = np.ascontiguousarray(cst)
    return out


def kernel(**inputs):
    import os
    inputs = {k: np.asarray(v, dtype=np.float32) for k, v in inputs.items()}
    if "prog" not in _CACHE:
        _CACHE["prog"] = _build_program()
    nc = _CACHE["prog"]

    in_maps = []
    for core in range(8):
        s, b = core // 4, core % 4
        if s == 0:
            x = inputs["g_x"][b]
            rb = (inputs["e_conv_w"], inputs["e_bn_g"], inputs["e_bn_b"], inputs["e_skip_w"])
            lng, lnb = inputs["ln1_g"], inputs["ln1_b"]
        else:
            x = inputs["r_x"][b]
            rb = (inputs["g_conv_w"], inputs["g_bn_g"], inputs["g_bn_b"], inputs["g_skip_w"])
            lng, lnb = inputs["ln2_g"], inputs["ln2_b"]
        in_maps.append(_prep_core_inputs(x, *rb, inputs, lng, lnb))

    kw = {}
    if os.environ.get("KERNEL_TRACE"):
        kw = dict(trace=True, tmpdir=os.environ.get("KERNEL_TRACE_DIR") or None)
    res = run_bass_kernel_spmd(nc, in_maps, list(range(8)), **kw)
    _CACHE["last_result"] = res
    g_out = np.stack([res.results[b]["y_out"] for b in range(4)]).astype(np.float32)
    r_out = np.stack([res.results[4 + b]["y_out"] for b in range(4)]).astype(np.float32)
    return g_out, r_out


# revision 36
# speedup vs baseline: 1.0088x; 1.0088x over previous
"""Trainium2 Bass kernel for the CoSSM block (bidirectional Mamba on two streams).

Sharding: 8 cores = 2 streams x 4 batches; each core runs the full pipeline for
its (stream, batch) slice.  Weights are replicated; the per-core input map
carries the stream-specific resbranch/LN weights.

On-chip layout is channel-major ([channels(partitions) x L(free)]).

v3 design notes (engine balance):
- fp16 scan pipeline: dbu / C-mult run as packed 2x-mode TensorTensor on DVE,
  the scans run packed (4 x 513 segments with zero reset columns) mostly on
  the Pool (gpsimd) engine with a slice diverted to DVE for balance.
- The n-reduction (sum of 16 state contributions) and the u*D skip term ride
  the PE: identity / diag(D) matmuls accumulating into one PSUM bank.
- B/C state-row broadcasts go DRAM->SBUF via stride-0-partition DMA.
- in_proj / dwconv / out_proj weights are fp16 (1 PE cycle/row).
- da = exp(-n*delta) on Act; da_2 = da_1^2 on DVE to balance Act/DVE.
- Small per-core constants ride in one packed DMA; activation-table choice is
  patched so Exp and Ln share one table (no per-dc table thrash).
"""

import sys
import types
import numpy as np

for _p in ("/opt/trn_rl_repo",):
    if _p not in sys.path:
        sys.path.append(_p)

import concourse.mybir as mybir
from concourse import bacc
from concourse.tile import TileContext
from concourse.bass_utils import run_bass_kernel_spmd

F32 = mybir.dt.float32
F16 = mybir.dt.float16
F32R = mybir.dt.float32r
AL = mybir.AluOpType
AF = mybir.ActivationFunctionType
AX = mybir.AxisListType

B, L = 4, 512
D_IN, D_MODEL = 256, 1024
D_INNER, D_STATE, D_CONV, DT_RANK = 2048, 16, 4, 64
BN_EPS, LN_EPS = 1e-5, 1e-6

NDC = D_INNER // 128          # 16 channel chunks of d_inner
NMC = D_MODEL // 128          # 8 channel chunks of d_model
NG = 4                        # states packed per scan instruction
NGRP = D_STATE // NG          # 4 groups
SEG = L + 1                   # segment length incl. reset column

# consts tile column layout
C_RBB, C_LNG, C_LNB = 0, 8, 16
C_CB, C_DTB, C_DV = 24, 56, 88
C_AN = 120                    # + br*256 + dc*16 + n   (512 cols)
C_TOT = 632

SCAN_DVE_MOD = 12             # every 12th (dc,g) scan runs on DVE instead of Pool

_CACHE = {}


def _patch_act_tables(nc):
    """Make Exp and Ln resolve to the combined exp+ln activation table so the
    per-dc Exp/Ln alternation doesn't reload tables 60+ times."""
    from concourse.hw_specs import get_activation_tables
    import bass_rust as _bass_rust

    def insert_act_table_loads(self):
        has_activation = any(
            isinstance(i, mybir.InstActivation)
            for b in self.main_func.blocks
            for i in b.instructions
        )
        if not has_activation:
            return
        tables = []
        items = list(get_activation_tables(self.m.arch).items())
        combined = {
            name for name, s in items
            if AF.Exp in s and AF.Ln in s
        }
        for name, s in items:
            if name not in combined and (AF.Exp in s) != (AF.Ln in s):
                s = s - {AF.Exp, AF.Ln}
            tables.append((name, s))
        _bass_rust.insert_act_table_loads(self, tables)

    nc.insert_act_table_loads = types.MethodType(insert_act_table_loads, nc)


def _build_program():
    nc = bacc.Bacc("TRN2", target_bir_lowering=False, debug=False)
    _patch_act_tables(nc)

    def din(name, shape, dt=F32):
        return nc.dram_tensor(name, list(shape), dt, kind="ExternalInput")

    d_x = din("x", [4, 128, D_IN], F32R)
    d_identr = din("identr", [128, 128], F32R)
    d_identf = din("identf", [128, 128], F32)
    d_identh = din("identh", [128, 128], F16)
    d_ones = din("ones", [1, 128], F32R)
    d_onescol = din("onescol", [128, 1], F16)
    d_rbw = din("rbw", [6, 128, D_MODEL], F32R)       # (k,cc) major
    d_rbskip = din("rbskip", [2, 128, D_MODEL], F32R)
    d_wproj = din("wproj", [8, 128, 2 * D_INNER], F16)
    d_convd = din("convd", [2, 4, 128, 4 * 512], F16)  # 4-dc batches
    d_xpw = din("xpw", [2, NDC, 128, 112], F16)
    d_dtw = din("dtw", [2, 2, 64, 1024], F32R)         # per-branch halves
    d_opw = din("opw", [NDC, 128, D_MODEL], F16)
    d_dvdiag = din("dvdiag", [128, 2 * NDC * 128], F16)
    d_consts = din("consts", [128, C_TOT], F32)

    d_out = nc.dram_tensor("y_out", [L, D_MODEL], F16, kind="ExternalOutput")
    # DRAM spill buffers (per-core scratch)
    d_x1 = nc.dram_tensor("x1_spill", [NMC, 128, L], F16)
    d_z = nc.dram_tensor("z_spill", [NDC, 128, L], F16)
    d_bc = nc.dram_tensor("bc_spill", [2, 2, D_STATE, L], F16)

    with TileContext(nc) as tc:
        with tc.tile_pool(name="sb", bufs=1) as sb, \
             tc.tile_pool(name="wp", bufs=3) as wp, \
             tc.tile_pool(name="tp", bufs=2) as tp, \
             tc.tile_pool(name="ps", bufs=8, space="PSUM") as ps:

            def psum():
                return ps.tile([128, 512], F32, tag="mm", name="mm")

            # ---- constants -------------------------------------------------
            t_cst = sb.tile([128, C_TOT], F32, tag="cst", name="cst")
            nc.sync.dma_start(out=t_cst[:], in_=d_consts[:])

            def an(br, dc, n):
                c = C_AN + br * 256 + dc * 16 + n
                return t_cst[:, c:c + 1]

            t_idr = sb.tile([128, 128], F32R, tag="idr", name="idr")
            nc.sync.dma_start(out=t_idr[:], in_=d_identr[:])
            t_idf = sb.tile([128, 128], F32, tag="idf", name="idf")
            nc.sync.dma_start(out=t_idf[:], in_=d_identf[:])
            t_idh = sb.tile([128, 128], F16, tag="idh", name="idh")
            nc.sync.dma_start(out=t_idh[:], in_=d_identh[:])
            t_ones = sb.tile([1, 128], F32R, tag="ones", name="ones")
            nc.sync.dma_start(out=t_ones[:], in_=d_ones[:])
            t_onescol = sb.tile([128, 1], F16, tag="onescol", name="onescol")
            nc.sync.dma_start(out=t_onescol[:], in_=d_onescol[:])
            t_dgall = sb.tile([128, 2 * NDC * 128], F16, tag="dgall", name="dgall")
            nc.sync.dma_start(out=t_dgall[:], in_=d_dvdiag[:])

            # ---- phase A: load x, transpose to [c, l] ----------------------
            t_xpad = [sb.tile([128, 514], F32R, tag=f"xp{cc}", name=f"xp{cc}") for cc in range(2)]
            for cc in range(2):
                nc.vector.memset(t_xpad[cc][:].bitcast(F32), 0.0)
            for i in range(4):
                xt = tp.tile([128, D_IN], F32R, tag="xinz", name="xin", bufs=2)
                nc.sync.dma_start(out=xt[:], in_=d_x[i])
                for cc in range(2):
                    pt = ps.tile([128, 128], F32R, tag="mm", name="mm")
                    nc.tensor.transpose(pt[:], xt[:, cc * 128:(cc + 1) * 128], t_idr[:])
                    nc.scalar.copy(t_xpad[cc][:, 1 + i * 128:1 + (i + 1) * 128], pt[:])

            # ---- phase B: resbranch -> x1 (fp16) ---------------------------
            # conv3 pass: 8 psum accumulators, one weight tile live at a time
            t_x1 = [sb.tile([128, L], F16, tag=f"x1_{m}", name=f"x1_{m}") for m in range(NMC)]
            pts = [psum() for _ in range(NMC)]
            for kc in range(6):
                k, cc = kc // 2, kc % 2
                wt = wp.tile([128, 1024], F32R, tag="wbig", name="wbig", bufs=4)
                nc.sync.dma_start(out=wt[:], in_=d_rbw[kc])
                for m in range(NMC):
                    nc.tensor.matmul(pts[m][:], wt[:, m * 128:(m + 1) * 128],
                                     t_xpad[cc][:, k:k + 512],
                                     start=(kc == 0), stop=(kc == 5))
            for m in range(NMC):
                nc.scalar.activation(t_x1[m][:], pts[m][:], AF.Relu,
                                     bias=t_cst[:, C_RBB + m:C_RBB + m + 1])
            # 1x1 skip pass, then add in place
            pts2 = [psum() for _ in range(NMC)]
            for cc in range(2):
                wt = wp.tile([128, 1024], F32R, tag="wbig", name="wbig", bufs=4)
                nc.sync.dma_start(out=wt[:], in_=d_rbskip[cc])
                for m in range(NMC):
                    nc.tensor.matmul(pts2[m][:], wt[:, m * 128:(m + 1) * 128],
                                     t_xpad[cc][:, 1:513],
                                     start=(cc == 0), stop=(cc == 1))
            for m in range(NMC):
                nc.vector.tensor_tensor(out=t_x1[m][:], in0=t_x1[m][:], in1=pts2[m][:], op=AL.add)
                nc.sync.dma_start(out=d_x1[m], in_=t_x1[m][:])

            # ---- phase C: in_proj (fp16) -> xi_pad fp16, z spill fp16 ------
            t_xi = [sb.tile([128, 518], F16, tag=f"xi{dc}", name=f"xi{dc}") for dc in range(NDC)]
            for dc in range(NDC):
                nc.vector.memset(t_xi[dc][:].bitcast(F32), 0.0)
            t_u = [[None] * NDC, [None] * NDC]

            def inproj_group(g, zcopy_pool=False):
                pts = [psum() for _ in range(8)]
                for kc in range(NMC):
                    wt = wp.tile([128, 1024], F16, tag="wbig", name="wbig", bufs=4)
                    nc.sync.dma_start(out=wt[:], in_=d_wproj[kc][:, g * 1024:(g + 1) * 1024])
                    for mj in range(8):
                        nc.tensor.matmul(pts[mj][:], wt[:, mj * 128:(mj + 1) * 128],
                                         t_x1[kc][:], start=(kc == 0), stop=(kc == NMC - 1))
                for mj in range(8):
                    mm = g * 8 + mj
                    if mm < NDC:
                        nc.scalar.copy(t_xi[mm][:, 3:515], pts[mj][:])
                    else:
                        zt = tp.tile([128, L], F16, tag="ztmp", name="ztmp")
                        nc.scalar.activation(zt[:], pts[mj][:], AF.Silu)
                        nc.sync.dma_start(out=d_z[mm - NDC], in_=zt[:])

            def dwconv_batch(br, dq):
                # depthwise conv + silu -> u (fp16) for dc in [4*dq, 4*dq+4)
                cdt4 = wp.tile([128, 2048], F16, tag="wbig", name="cdt4", bufs=4)
                nc.sync.dma_start(out=cdt4[:], in_=d_convd[br, dq])
                for dj in range(4):
                    dc = dq * 4 + dj
                    pu = psum()
                    for k in range(4):
                        if br == 0:
                            rhs = t_xi[dc][:, k:k + 512]
                        else:
                            rhs = t_xi[dc][:, 6 - k:518 - k][:, ::-1]
                        nc.tensor.matmul(pu[:], cdt4[:, dj * 512 + k * 128:dj * 512 + (k + 1) * 128],
                                         rhs, start=(k == 0), stop=(k == 3))
                    ut = sb.tile([128, L], F16, tag=f"u{br}_{dc}", name=f"u{br}_{dc}")
                    nc.scalar.activation(ut[:], pu[:], AF.Silu,
                                         bias=t_cst[:, C_CB + dc * 2 + br:C_CB + dc * 2 + br + 1])
                    t_u[br][dc] = ut

            # ---- phases C+D interleaved: in_proj groups feed dwconv early --
            inproj_group(0)
            for br in range(2):
                for dq in (0, 1):
                    dwconv_batch(br, dq)
            inproj_group(1)
            for br in range(2):
                for dq in (2, 3):
                    dwconv_batch(br, dq)

            # y accumulators (fp16, in xi slots; filled per branch below)
            t_y = [sb.tile([128, L], F16, tag=f"xi{dc}", name=f"y{dc}") for dc in range(NDC)]

            # ---- phase E: selective scan per branch ------------------------
            t_Bb = [sb.tile([128, NG * L], F16, tag=f"Bb{g}", name=f"Bb{g}") for g in range(NGRP)]
            t_Cb = [sb.tile([128, NG * L], F16, tag=f"Cb{g}", name=f"Cb{g}") for g in range(NGRP)]
            n_da = [0]
            n_flex = [0]

            def pool_pick():
                # route 7 of every 16 dbu/q TensorTensors to the Pool ucode
                r = (n_flex[0] * 7) % 16 < 7
                n_flex[0] += 1
                return r

            for br in range(2):
                # x_proj: px [112, L] = dt rows 0:64, B rows 64:80, C rows 96:112
                px = ps.tile([112, 512], F32, tag="mm", name="px")
                for dh in range(2):
                    wx = wp.tile([128, 8 * 112], F16, tag="wxp", name="wxp", bufs=2)
                    nc.sync.dma_start(out=wx[:].rearrange("p (d c) -> p d c", d=8),
                                      in_=d_xpw[br, dh * 8:(dh + 1) * 8].transpose([1, 0, 2]))
                    for dj in range(8):
                        dc = dh * 8 + dj
                        nc.tensor.matmul(px[:], wx[:, dj * 112:(dj + 1) * 112],
                                         t_u[br][dc][:],
                                         start=(dc == 0), stop=(dc == NDC - 1))
                t_dtw = []
                for h in range(2):
                    wdt = wp.tile([64, 1024], F32R, tag="wdt", name="wdt", bufs=4)
                    nc.sync.dma_start(out=wdt[:], in_=d_dtw[br, h])
                    t_dtw.append(wdt)
                t_dtT = sb.tile([64, L], F32R, tag="xp0", name="dtT")
                nc.scalar.copy(t_dtT[:], px[0:64, :])
                t_Brow = tp.tile([D_STATE, L], F16, tag="Brow", name="Brow", bufs=1)
                nc.scalar.copy(t_Brow[:], px[64:80, :])
                t_Crow = tp.tile([D_STATE, L], F16, tag="Crow", name="Crow", bufs=1)
                nc.scalar.copy(t_Crow[:], px[96:112, :])
                nc.sync.dma_start(out=d_bc[br, 0], in_=t_Brow[:])
                nc.sync.dma_start(out=d_bc[br, 1], in_=t_Crow[:])
                for g in range(NGRP):
                    src_b = d_bc[br, 0, 4 * g:4 * g + 4].rearrange("n c -> (n c)").unsqueeze(0).to_broadcast([128, NG * L])
                    nc.sync.dma_start(out=t_Bb[g][:], in_=src_b)
                    src_c = d_bc[br, 1, 4 * g:4 * g + 4].rearrange("n c -> (n c)").unsqueeze(0).to_broadcast([128, NG * L])
                    nc.sync.dma_start(out=t_Cb[g][:], in_=src_c)

                def delta(dc):
                    # softplus(dt_proj) for chunk dc -> (d8, w8), fp16
                    pd = psum()
                    nc.tensor.matmul(pd[:], t_dtw[dc // 8][:, (dc % 8) * 128:(dc % 8 + 1) * 128],
                                     t_dtT[:], start=True, stop=True)
                    et = tp.tile([128, L], F16, tag="eth", name="et", bufs=4)
                    nc.scalar.activation(et[:], pd[:], AF.Exp,
                                         bias=t_cst[:, C_DTB + dc * 2 + br:C_DTB + dc * 2 + br + 1])
                    d8 = tp.tile([128, L], F16, tag="d8", name="d8", bufs=4)
                    nc.scalar.activation(d8[:], et[:], AF.Ln, bias=1.0)
                    w8 = tp.tile([128, L], F16, tag="w8", name="w8", bufs=4)
                    nc.vector.tensor_tensor(out=w8[:], in0=d8[:],
                                            in1=t_u[br][dc][:], op=AL.mult)
                    return d8, w8

                # Software-pipelined scan loop: scans are consumed (C-mult +
                # PE reduction) with a LAG of 2 (dc,g)-iterations so the DVE
                # always has dbu work queued ahead of q's that wait on Pool.
                LAG = 2
                paccs = {}
                pend = []

                def consume(hh0, dc0, g0):
                    q = tp.tile([128, NG * L], F16, tag="q", name="q", bufs=2)
                    eng = nc.gpsimd if (g0 == 0 and dc0 % 2 == 0) else nc.vector
                    eng.tensor_tensor(
                        out=q[:].rearrange("p (n c) -> p n c", n=NG),
                        in0=hh0[:].rearrange("p (n c) -> p n c", n=NG)[:, :, 1:SEG],
                        in1=t_Cb[g0][:].rearrange("p (n c) -> p n c", n=NG),
                        op=AL.mult)
                    for jn in range(NG):
                        qs = q[:, jn * L:(jn + 1) * L]
                        if br == 1:
                            qs = qs[:, ::-1]
                        nc.tensor.matmul(paccs[dc0][:], t_idh[:], qs,
                                         start=False,
                                         stop=(g0 == NGRP - 1 and jn == NG - 1))
                    if g0 == NGRP - 1:
                        if br == 0:
                            nc.vector.tensor_copy(out=t_y[dc0][:], in_=paccs[dc0][:])
                        else:
                            nc.vector.tensor_tensor(out=t_y[dc0][:], in0=t_y[dc0][:],
                                                    in1=paccs[dc0][:], op=AL.add)
                        del paccs[dc0]

                dq_delta = [delta(0), delta(1), delta(2)]
                # z-half of in_proj rides the pipeline-fill windows (PE slack);
                # silu is applied at copy time so phase F only multiplies.
                inproj_group(2 + br)
                for dc in range(NDC):
                    d8, w8 = dq_delta.pop(0)
                    # PSUM accumulator for this (br, dc): u*D skip + 16 state slices
                    pacc = psum()
                    paccs[dc] = pacc
                    mv = t_u[br][dc][:] if br == 0 else t_u[br][dc][:, ::-1]
                    nc.tensor.matmul(pacc[:], t_dgall[:, (br * NDC + dc) * 128:(br * NDC + dc + 1) * 128],
                                     mv, start=True, stop=False)
                    for g in range(NGRP):
                        if dc + 3 < NDC and g == 2:
                            dq_delta.append(delta(dc + 3))
                        da = tp.tile([128, NG * SEG], F16, tag="da", name="da", bufs=2)
                        dbu = tp.tile([128, NG * SEG], F16, tag="dbu", name="dbu", bufs=3)
                        if n_da[0] < 3:
                            if n_da[0] < 2:
                                nc.vector.memset(da[:].bitcast(F32), 0.0)
                            nc.vector.memset(dbu[:].bitcast(F32), 0.0)
                        for jn in range(NG):
                            n = g * NG + jn
                            nc.scalar.activation(
                                da[:, jn * SEG + 1:(jn + 1) * SEG], d8[:],
                                AF.Exp, scale=an(br, dc, n))
                        dbu_sl = dbu[:].rearrange("p (n c) -> p n c", n=NG)[:, :, 1:SEG]
                        eng = nc.gpsimd if g < 3 else nc.vector
                        eng.tensor_tensor(
                            out=dbu_sl,
                            in0=w8[:].unsqueeze(1).to_broadcast([128, NG, L]),
                            in1=t_Bb[g][:].rearrange("p (n c) -> p n c", n=NG),
                            op=AL.mult)
                        hh = tp.tile([128, NG * SEG], F16, tag="hh", name="hh", bufs=3)
                        nc.vector.tensor_tensor_scan(hh[:], da[:], dbu[:], 0.0,
                                                     AL.mult, AL.add)
                        n_da[0] += 1
                        pend.append((hh, dc, g))
                        if len(pend) > LAG:
                            consume(*pend.pop(0))
                while pend:
                    consume(*pend.pop(0))

            # ---- phase F: gate, out_proj, layernorm, residual --------------
            for dc in range(NDC):
                zt = tp.tile([128, L], F16, tag="ztmp", name="ztmp")
                nc.sync.dma_start(out=zt[:], in_=d_z[dc])
                nc.vector.tensor_tensor(out=t_y[dc][:], in0=t_y[dc][:], in1=zt[:], op=AL.mult)

            t_o1 = [sb.tile([128, L], F16, tag=f"u0_{m}", name=f"o1_{m}") for m in range(NMC)]
            pos = [psum() for _ in range(NMC)]
            for dp in range(NDC // 2):
                wt = wp.tile([128, 2048], F16, tag="wbig", name="wbig", bufs=4)
                nc.sync.dma_start(out=wt[:].rearrange("p (d c) -> p d c", d=2),
                                  in_=d_opw[2 * dp:2 * dp + 2].transpose([1, 0, 2]))
                for dj in range(2):
                    dc = 2 * dp + dj
                    for m in range(NMC):
                        nc.tensor.matmul(pos[m][:], wt[:, dj * 1024 + m * 128:dj * 1024 + (m + 1) * 128],
                                         t_y[dc][:], start=(dc == 0), stop=(dc == NDC - 1))
            for m in range(NMC):
                nc.scalar.copy(t_o1[m][:], pos[m][:])

            # layernorm stats via column-sum matmuls
            pm = ps.tile([1, 512], F32, tag="mm", name="pm")
            for m in range(NMC):
                nc.tensor.matmul(pm[:], t_onescol[:], t_o1[m][:],
                                 start=(m == 0), stop=(m == NMC - 1))
            pq = ps.tile([1, 512], F32, tag="mm", name="pq")
            for m in range(NMC):
                sq = tp.tile([128, L], F16, tag="ztmp", name="sq")
                nc.scalar.activation(sq[:], t_o1[m][:], AF.Square)
                nc.tensor.matmul(pq[:], t_onescol[:], sq[:],
                                 start=(m == 0), stop=(m == NMC - 1))
            t_mean = sb.tile([1, L], F32R, tag="mean", name="mean")
            nc.scalar.activation(t_mean[:], pm[:], AF.Copy, scale=1.0 / D_MODEL)
            t_var = tp.tile([1, L], F32, tag="et", name="stat")
            nc.scalar.activation(t_var[:], pq[:], AF.Copy, scale=1.0 / D_MODEL)
            msq = tp.tile([1, L], F32, tag="et", name="msq")
            nc.vector.tensor_tensor(out=msq[:], in0=t_mean[:], in1=t_mean[:], op=AL.mult)
            nc.vector.tensor_tensor(out=t_var[:], in0=t_var[:], in1=msq[:], op=AL.subtract)
            t_eps = sb.tile([1, 1], F32, tag="eps", name="eps")
            nc.vector.memset(t_eps[:], LN_EPS)
            t_sd = tp.tile([1, L], F32, tag="q", name="stat2", bufs=2)
            nc.scalar.activation(t_sd[:], t_var[:], AF.Sqrt, bias=t_eps[:])
            t_isd = sb.tile([1, L], F32R, tag="isd", name="isd")
            with nc.allow_low_precision(reason="isd is a broadcast-matmul rhs"):
                nc.vector.reciprocal(out=t_isd[:], in_=t_sd[:])
            # broadcast mean, isd
            pmb = psum()
            nc.tensor.matmul(pmb[:], t_ones[:], t_mean[:], start=True, stop=True)
            t_mb = sb.tile([128, L], F32, tag="Bb0", name="mb")
            nc.scalar.copy(t_mb[:], pmb[:])
            pib = psum()
            nc.tensor.matmul(pib[:], t_ones[:], t_isd[:], start=True, stop=True)
            t_ib = sb.tile([128, L], F32, tag="Bb1", name="ib")
            nc.scalar.copy(t_ib[:], pib[:])

            t_of = []
            for m in range(NMC):
                x1r = tp.tile([128, L], F16, tag="d8", name="x1r", bufs=4)
                nc.sync.dma_start(out=x1r[:], in_=d_x1[m])
                tt = tp.tile([128, L], F32, tag="et", name="ft")
                nc.vector.tensor_tensor(out=tt[:], in0=t_o1[m][:], in1=t_mb[:], op=AL.subtract)
                nc.vector.tensor_tensor(out=tt[:], in0=tt[:], in1=t_ib[:], op=AL.mult)
                nc.vector.tensor_scalar(out=tt[:], in0=tt[:],
                                        scalar1=t_cst[:, C_LNG + m:C_LNG + m + 1],
                                        scalar2=t_cst[:, C_LNB + m:C_LNB + m + 1],
                                        op0=AL.mult, op1=AL.add)
                ot = sb.tile([128, L], F16, tag=f"u1_{m}", name=f"of_{m}")
                nc.vector.tensor_tensor(out=ot[:], in0=tt[:], in1=x1r[:], op=AL.add)
                t_of.append(ot)

            # transpose back to [l, d] and store
            for i in range(4):
                outt = wp.tile([128, D_MODEL], F16, tag="wbig", name="outt", bufs=4)
                for m in range(NMC):
                    ptr = ps.tile([128, 128], F16, tag="mm", name="ptr")
                    nc.tensor.transpose(ptr[:], t_of[m][:, i * 128:(i + 1) * 128], t_idh[:])
                    nc.vector.tensor_copy(out=outt[:, m * 128:(m + 1) * 128], in_=ptr[:])
                nc.sync.dma_start(out=d_out[i * 128:(i + 1) * 128, :], in_=outt[:])

    nc.compile()
    return nc


def _prep_core_inputs(x, rb_conv_w, rb_bn_g, rb_bn_b, rb_skip_w, inp, ln_g, ln_b):
    f32 = np.float32
    f16 = np.float16
    out = {}
    out["x"] = np.ascontiguousarray(x.reshape(4, 128, D_IN)).astype(f32)
    out["identr"] = np.eye(128, dtype=f32)
    out["identf"] = np.eye(128, dtype=f32)
    out["identh"] = np.eye(128, dtype=f16)
    out["ones"] = np.ones((1, 128), f32)
    out["onescol"] = np.ones((128, 1), f16)
    s = f32(1.0) / np.sqrt(np.float64(1.0 + BN_EPS))
    Wc = (rb_conv_w * (rb_bn_g * s)[:, None, None]).astype(f32)   # [1024,256,3]
    rbw = np.transpose(Wc, (2, 1, 0)).reshape(6, 128, D_MODEL)
    out["rbw"] = np.ascontiguousarray(rbw)
    rbs = rb_skip_w[:, :, 0].T.reshape(2, 128, D_MODEL)           # [c, m]
    out["rbskip"] = np.ascontiguousarray(rbs.astype(f32))
    out["wproj"] = np.ascontiguousarray(inp["in_proj_w"].T.reshape(8, 128, 2 * D_INNER).astype(f16))
    convd = np.zeros((2, NDC, 128, 4, 128), f16)
    r = np.arange(128)
    for br, key in enumerate(["conv_w_f", "conv_w_b"]):
        cw = inp[key].astype(f16)
        for dc in range(NDC):
            for k in range(4):
                convd[br, dc, r, k, r] = cw[dc * 128:(dc + 1) * 128, k]
    out["convd"] = np.ascontiguousarray(convd.reshape(2, 4, 4, 128, 512).transpose(0, 1, 3, 2, 4).reshape(2, 4, 128, 2048))
    xpw = np.zeros((2, D_INNER, 112), np.float32)
    for br, key in enumerate(["x_proj_f", "x_proj_b"]):
        xp = inp[key].T  # [2048, 96]
        xpw[br, :, 0:80] = xp[:, 0:80]
        xpw[br, :, 96:112] = xp[:, 80:96]
    out["xpw"] = np.ascontiguousarray(xpw.reshape(2, NDC, 128, 112).astype(f16))
    out["dtw"] = np.ascontiguousarray(np.stack(
        [inp["dt_w_f"].T, inp["dt_w_b"].T]).reshape(2, 64, 2, 1024).transpose(0, 2, 1, 3).astype(f32))
    out["opw"] = np.ascontiguousarray(inp["out_proj_w"].T.reshape(NDC, 128, D_MODEL).astype(f16))
    dvd = np.zeros((2, NDC, 128, 128), f16)
    for br, key in enumerate(["D_f", "D_b"]):
        dv = inp[key].astype(f16).reshape(NDC, 128)
        for dc in range(NDC):
            dvd[br, dc, r, r] = dv[dc]
    out["dvdiag"] = np.ascontiguousarray(dvd.transpose(2, 0, 1, 3).reshape(128, 2 * NDC * 128))

    cst = np.zeros((128, C_TOT), f32)
    cst[:, C_RBB:C_RBB + 8] = rb_bn_b.reshape(NMC, 128).T
    cst[:, C_LNG:C_LNG + 8] = ln_g.reshape(NMC, 128).T
    cst[:, C_LNB:C_LNB + 8] = ln_b.reshape(NMC, 128).T
    cb = np.stack([inp["conv_bias_f"].reshape(NDC, 128),
                   inp["conv_bias_b"].reshape(NDC, 128)], -1)     # [16,128,2]
    cst[:, C_CB:C_CB + 32] = cb.transpose(1, 0, 2).reshape(128, 32)
    dtb = np.stack([inp["dt_bias_f"].reshape(NDC, 128),
                    inp["dt_bias_b"].reshape(NDC, 128)], -1)
    cst[:, C_DTB:C_DTB + 32] = dtb.transpose(1, 0, 2).reshape(128, 32)
    dvv = np.stack([inp["D_f"].reshape(NDC, 128), inp["D_b"].reshape(NDC, 128)], -1)
    cst[:, C_DV:C_DV + 32] = dvv.transpose(1, 0, 2).reshape(128, 32)
    aneg = np.stack([-np.exp(inp["A_log_f"]), -np.exp(inp["A_log_b"])])  # [2,2048,16]
    cst[:, C_AN:C_AN + 512] = aneg.reshape(2, NDC, 128, 16).transpose(2, 0, 1, 3).reshape(128, 512)
    out["consts"] = np.ascontiguousarray(cst)
    return out


def kernel(**inputs):
    import os
    inputs = {k: np.asarray(v, dtype=np.float32) for k, v in inputs.items()}
    if "prog" not in _CACHE:
        _CACHE["prog"] = _build_program()
    nc = _CACHE["prog"]

    in_maps = []
    for core in range(8):
        s, b = core // 4, core % 4
        if s == 0:
            x = inputs["g_x"][b]
            rb = (inputs["e_conv_w"], inputs["e_bn_g"], inputs["e_bn_b"], inputs["e_skip_w"])
            lng, lnb = inputs["ln1_g"], inputs["ln1_b"]
        else:
            x = inputs["r_x"][b]
            rb = (inputs["g_conv_w"], inputs["g_bn_g"], inputs["g_bn_b"], inputs["g_skip_w"])
            lng, lnb = inputs["ln2_g"], inputs["ln2_b"]
        in_maps.append(_prep_core_inputs(x, *rb, inputs, lng, lnb))

    kw = {}
    if os.environ.get("KERNEL_TRACE"):
        kw = dict(trace=True, tmpdir=os.environ.get("KERNEL_TRACE_DIR") or None)
    res = run_bass_kernel_spmd(nc, in_maps, list(range(8)), **kw)
    _CACHE["last_result"] = res
    g_out = np.stack([res.results[b]["y_out"] for b in range(4)]).astype(np.float32)
    r_out = np.stack([res.results[4 + b]["y_out"] for b in range(4)]).astype(np.float32)
    return g_out, r_out
